# revision 6
# baseline (speedup 1.0000x reference)
"""ASTGCN block kernel for Trainium2 (8 NeuronCores, SPMD data-parallel over batch).

Strategy: the Chebyshev graph convolution (sum_k (cheb_k * S)^T @ xTheta_k,
a (1024 x 3072) @ (3072 x 4096) matmul per batch = ~85% of all FLOPs) runs
on-device in bf16 via a Bass/Tile kernel, data-parallel over batch across the
8 cores (2 batches/core).  The small attention matmuls (producing S) and the
elementwise epilogue run on host in fp32.
"""

import numpy as np
import ml_dtypes
from contextlib import ExitStack

import concourse.bass as bass
import concourse.bacc as bacc
import concourse.mybir as mybir
from concourse import tile
from concourse.bass_utils import run_bass_kernel_spmd

B, N, Fh, T, K, C = 16, 1024, 64, 64, 3, 64
NCORES = 8
BLOC = B // NCORES  # 2 batches per core
KM = K * N          # 3072 contraction length
CT = C * T          # 4096 output free length

_cached = {}
_last_in_maps = None


def _build_nc():
    if "nc" in _cached:
        return _cached["nc"]
    nc = bacc.Bacc("TRN2", target_bir_lowering=False, debug=False)
    bf16 = mybir.dt.bfloat16
    f32 = mybir.dt.float32
    a_d = nc.declare_dram_parameter("A", [BLOC, KM, N], bf16, isOutput=False)
    x_d = nc.declare_dram_parameter("XT", [BLOC, KM, CT], bf16, isOutput=False)
    o_d = nc.declare_dram_parameter("OUT", [BLOC, N, CT], f32, isOutput=True)

    NKC = KM // 128   # 24 contraction chunks
    NNT = N // 128    # 8 output row tiles
    NOC = CT // 512   # 8 output col chunks

    with tile.TileContext(nc) as tc, ExitStack() as ctx:
        apool = ctx.enter_context(tc.tile_pool(name="a", bufs=1))
        xpool = ctx.enter_context(tc.tile_pool(name="x", bufs=3))
        opool = ctx.enter_context(tc.tile_pool(name="o", bufs=4))
        pspool = ctx.enter_context(
            tc.tile_pool(name="ps", bufs=4, space=bass.MemorySpace.PSUM)
        )
        a_r = a_d.rearrange("b (kc p) n -> b p kc n", p=128)
        x_r = x_d.rearrange("b (kc p) ct -> b p kc ct", p=128)
        for b in range(BLOC):
            asb = apool.tile([128, NKC, N], bf16, tag="a")
            nc.sync.dma_start(asb[:], a_r[b])
            for oc in range(NOC):
                xsb = xpool.tile([128, NKC, 512], bf16, tag="x")
                nc.sync.dma_start(xsb[:], x_r[b, :, :, oc * 512 : (oc + 1) * 512])
                for nt in range(NNT):
                    ps = pspool.tile([128, 512], f32, tag="ps")
                    for kc in range(NKC):
                        nc.tensor.matmul(
                            ps[:],
                            asb[:, kc, nt * 128 : (nt + 1) * 128],
                            xsb[:, kc, :],
                            start=(kc == 0),
                            stop=(kc == NKC - 1),
                        )
                    osb = opool.tile([128, 512], f32, tag="o")
                    nc.vector.tensor_copy(osb[:], ps[:])
                    nc.sync.dma_start(
                        o_d[b, nt * 128 : (nt + 1) * 128, oc * 512 : (oc + 1) * 512],
                        osb[:],
                    )
    nc.compile()
    _cached["nc"] = nc
    return nc


def _softmax_ax1(s):
    m = s.max(axis=1, keepdims=True)
    e = np.exp(s - m)
    return e / e.sum(axis=1, keepdims=True)


def _sigmoid(x):
    return 1.0 / (1.0 + np.exp(-x))


def kernel(x, cheb, Theta, W1, W2, W3, b_s, V_s, U1, U2, U3, b_e, V_e,
           tw, tb, rw, rb, gamma, beta):
    f32 = np.float32
    x = np.asarray(x, f32)
    cheb = np.asarray(cheb, f32)
    Theta = np.asarray(Theta, f32)

    # ---- temporal attention (host, small) ----
    xu1 = np.einsum("bnft,n->btf", x, np.asarray(U1, f32), optimize=True)  # (B,T,F)
    lhs = xu1 @ np.asarray(U2, f32)                                        # (B,T,N)
    rhs = np.einsum("f,bnft->bnt", np.asarray(U3, f32), x, optimize=True)  # (B,N,T)
    prod = np.matmul(lhs, rhs)                                             # (B,T,T)
    sig = _sigmoid(prod + np.asarray(b_e, f32))
    E = np.einsum("kj,bij->bik", np.asarray(V_e, f32), sig, optimize=True)
    E = _softmax_ax1(E)
    x_tat = np.einsum("bnfj,bjt->bnft", x, E, optimize=True)

    # ---- spatial attention (host) ----
    lhs2 = np.einsum("bnft,t->bnf", x_tat, np.asarray(W1, f32), optimize=True) @ \
        np.asarray(W2, f32)                                                # (B,N,T)
    rhs2 = np.einsum("f,bnft->btn", np.asarray(W3, f32), x_tat, optimize=True)
    prod2 = np.matmul(lhs2, rhs2)                                          # (B,N,N)
    sig2 = _sigmoid(prod2 + np.asarray(b_s, f32))
    S = np.einsum("kj,bij->bik", np.asarray(V_s, f32), sig2, optimize=True)
    S = _softmax_ax1(S)                                                    # (B,N,N)

    # ---- device operands for the big graph-conv matmul ----
    # A[b,(k,m),n] = cheb[k,m,n] * S[b,m,n]
    A = (cheb[None, :, :, :] * S[:, None, :, :]).reshape(B, KM, N)
    # XT[b,(k,m),(o,t)] = sum_f x[b,m,f,t] * Theta[k,f,o]
    xbtf = np.ascontiguousarray(x.transpose(0, 1, 3, 2)).reshape(B * N * T, Fh)
    xt_parts = []
    for k in range(K):
        p = (xbtf @ Theta[k]).reshape(B, N, T, C).transpose(0, 1, 3, 2)  # (B,N,C,T)
        xt_parts.append(p.reshape(B, N, CT))
    XT = np.stack(xt_parts, axis=1).reshape(B, KM, CT)

    A16 = A.astype(ml_dtypes.bfloat16)
    XT16 = XT.astype(ml_dtypes.bfloat16)

    nc = _build_nc()
    in_maps = [
        {"A": A16[c * BLOC : (c + 1) * BLOC], "XT": XT16[c * BLOC : (c + 1) * BLOC]}
        for c in range(NCORES)
    ]
    global _last_in_maps
    _last_in_maps = in_maps
    res = run_bass_kernel_spmd(nc, in_maps, core_ids=list(range(NCORES)))
    out = np.concatenate([r["OUT"] for r in res.results], axis=0)  # (B,N,CT)
    out = out.reshape(B, N, C, T)

    # ---- epilogue (host): relu, temporal conv, residual, layernorm ----
    sg = np.maximum(out, 0.0)
    tw = np.asarray(tw, f32)  # (C, C, 1, 3) OIHW
    tc_out = np.einsum("bnit,oi->bnot", sg, tw[:, :, 0, 1], optimize=True)
    t0 = np.einsum("bnit,oi->bnot", sg[:, :, :, :-1], tw[:, :, 0, 0], optimize=True)
    tc_out[:, :, :, 1:] += t0
    t2 = np.einsum("bnit,oi->bnot", sg[:, :, :, 1:], tw[:, :, 0, 2], optimize=True)
    tc_out[:, :, :, :-1] += t2
    tc_out += np.asarray(tb, f32)[None, None, :, None]

    res_c = np.einsum("bnft,of->bnot", x, np.asarray(rw, f32), optimize=True)
    res_c += np.asarray(rb, f32)[None, None, :, None]

    y = np.maximum(res_c + tc_out, 0.0)
    mu = y.mean(axis=-1, keepdims=True)
    var = y.var(axis=-1, keepdims=True)
    yn = (y - mu) / np.sqrt(var + 1e-5)
    # gamma/beta are (64,) and broadcast along the LAST axis (T), as in reference
    yn = yn * np.asarray(gamma, f32) + np.asarray(beta, f32)
    return yn.astype(np.float32)
